# revision 22
# baseline (speedup 1.0000x reference)
"""Causal GQA self-attention (B=2, S=2048, D=2048, H=16, KV=4) on 8 TRN2 cores.

Sharding: core = (b, g) with b = batch (2) x g = kv-head group (4).
Each core computes 4 q-heads / 1 kv-head for one batch and a partial
projection output [S, D] in bf16; host sums the 4 group partials per batch.

v4 = v2 (3-phase, 303us) + targeted wins:
  - startup: group-0 QKV consumed ci-major in DMA arrival order (each
    weight/x chunk feeds 4 s-tiles), q_ps for si2/3 borrow the score psum
    slots, retuned 3-queue DMA order. Cuts ~15us of PE idle at startup.
  - row-sum matmuls removed (~26us of PE streaming): probs accumulated
    per-unit on DVE into a bf16 sbuf tile (diag blocks via one strided op),
    one small ones-matmul pair per unit; acc adds delayed one block so diag
    masks aren't queued behind them on DVE.
  - projection psum pairs alternate between B banks and an S slot
    (deeper rotation, no stall on the ev-copy latency); ev DMA alternates
    sync/gpsimd queues; 6 ev buffers.
"""
import os
import sys

if '/opt/trn_rl_repo' not in sys.path:
    sys.path.insert(0, '/opt/trn_rl_repo')

import numpy as np

B, S, D = 2, 2048, 2048
NH_TOT, NKV_TOT, HD = 16, 4, 128
NH = 4                 # q heads per core
NT = S // 128          # 16 s-tiles
NC_ = D // 128         # 16 c-tiles
T = 4                  # q-slices of 512
SM = 1.0 / np.sqrt(HD)
EPS = float(np.finfo(np.float32).eps)
ROPE_BASE = 10000.0

_PROG = None


def _build_program():
    import concourse.bass as bass
    import concourse.mybir as mybir
    import concourse.tile as tile
    from concourse import bacc
    from concourse.alu_op_type import AluOpType

    F32 = mybir.dt.float32
    BF16 = mybir.dt.bfloat16
    AF = mybir.ActivationFunctionType

    nc = bacc.Bacc("TRN2", target_bir_lowering=False, debug=False)

    XT = nc.dram_tensor("XT", [4, 4, 128, 4, 512], BF16, kind="ExternalInput")
    WQ = nc.dram_tensor("WQ", [4, 128, 4, 512], BF16, kind="ExternalInput")   # [c4, c_p, a, dq]
    WKV = nc.dram_tensor("WKV", [4, 128, 4, 256], BF16, kind="ExternalInput")
    WP = nc.dram_tensor("WP", [4, 128, NH, 512], BF16, kind="ExternalInput")  # [dq, c_in_head, h, dout]
    CS = nc.dram_tensor("CS", [2, 128, NT, HD], BF16, kind="ExternalInput")   # cos;sin
    GSM = nc.dram_tensor("GSM", [1, NH], F32, kind="ExternalInput")           # gain*sm per head
    CONST3 = nc.dram_tensor("CONST3", [128, 3, 128], BF16, kind="ExternalInput")  # ident|onesq|tri
    Y = nc.dram_tensor("Y", [NT, 2, 128, 1024], BF16, kind="ExternalOutput")

    with tile.TileContext(nc) as tc:
        with (
            tc.tile_pool(name="const", bufs=1) as const,
            tc.tile_pool(name="w", bufs=4) as wpool,
            tc.tile_pool(name="stream", bufs=3) as stream,
            tc.tile_pool(name="small", bufs=3) as small,
            tc.tile_pool(name="norm", bufs=4) as normp,
            tc.tile_pool(name="rope", bufs=2) as ropep,
            tc.tile_pool(name="qsb", bufs=4) as qsbp,
            tc.tile_pool(name="big", bufs=1) as big,
            tc.tile_pool(name="yt", bufs=2) as ytp,
            tc.tile_pool(name="probs", bufs=4) as probsp,
            tc.tile_pool(name="accp", bufs=2) as accp,
            tc.tile_pool(name="outsb", bufs=6) as outsb,
            tc.tile_pool(name="psA", bufs=2, space="PSUM") as psA,
            tc.tile_pool(name="psB", bufs=2, space="PSUM") as psB,
            tc.tile_pool(name="psS", bufs=2, space="PSUM") as psS,
        ):
            wq_sb = [wpool.tile([128, 4, 512], BF16, tag="wq", name=f"wq{i}")
                     for i in range(4)]
            wkv_sb = [wpool.tile([128, 4, 256], BF16, tag="wkv", name=f"wkv{i}")
                      for i in range(4)]
            wp = [wpool.tile([128, NH, 512], BF16, tag="wp", name=f"wp{i}")
                  for i in range(4)]
            xsp = [[stream.tile([128, 4, 512], BF16, tag="xs", bufs=12,
                                name=f"xs_{s4}_{c4}")
                    for c4 in range(4)] for s4 in range(4)]

            gsm = const.tile([1, NH], F32)
            const3 = const.tile([128, 3, 128], BF16)
            cs = const.tile([128, 2, NT, HD], BF16)

            # scalar queue: gsm, WQ0, WKV0, WQ2, WKV2, cs, const3, WP
            nc.scalar.dma_start(gsm[:], GSM[:])
            nc.scalar.dma_start(wq_sb[0][:], WQ[0])
            nc.scalar.dma_start(wkv_sb[0][:], WKV[0])
            nc.scalar.dma_start(wq_sb[2][:], WQ[2])
            nc.scalar.dma_start(wkv_sb[2][:], WKV[2])
            nc.scalar.dma_start(cs[:, 0], CS[0])
            nc.scalar.dma_start(cs[:, 1], CS[1])
            nc.scalar.dma_start(const3[:], CONST3[:])
            for dq in range(4):
                nc.scalar.dma_start(wp[dq][:], WP[dq])

            # sync queue: XT00, XT01, WQ3, WKV3, XT1 c0/c1, then Y duty
            nc.sync.dma_start(xsp[0][0][:], XT[0, 0])
            nc.sync.dma_start(xsp[0][1][:], XT[0, 1])
            nc.sync.dma_start(wq_sb[3][:], WQ[3])
            nc.sync.dma_start(wkv_sb[3][:], WKV[3])
            nc.sync.dma_start(xsp[1][0][:], XT[1, 0])
            nc.sync.dma_start(xsp[1][1][:], XT[1, 1])
            nc.sync.dma_start(xsp[2][2][:], XT[2, 2])
            nc.sync.dma_start(xsp[2][3][:], XT[2, 3])
            nc.sync.dma_start(xsp[3][2][:], XT[3, 2])
            nc.sync.dma_start(xsp[3][3][:], XT[3, 3])

            # gpsimd queue: WQ1, WKV1, XT02, XT03, XT1 c2/c3, XT2, XT3, Y duty
            nc.gpsimd.dma_start(wq_sb[1][:], WQ[1])
            nc.gpsimd.dma_start(wkv_sb[1][:], WKV[1])
            nc.gpsimd.dma_start(xsp[0][2][:], XT[0, 2])
            nc.gpsimd.dma_start(xsp[0][3][:], XT[0, 3])
            nc.gpsimd.dma_start(xsp[1][2][:], XT[1, 2])
            nc.gpsimd.dma_start(xsp[1][3][:], XT[1, 3])
            for c4 in range(2):
                nc.gpsimd.dma_start(xsp[2][c4][:], XT[2, c4])
            for c4 in range(2):
                nc.gpsimd.dma_start(xsp[3][c4][:], XT[3, c4])

            ident = const3[:, 0, :]
            onesq = const3[:, 1, :]
            tri = const3[:, 2, :]
            cos2 = cs[:, 0]
            sin2 = cs[:, 1]
            gsm_bc = const.tile([128, NH], F32)
            nc.gpsimd.partition_broadcast(gsm_bc[:], gsm[:])

            qT = big.tile([128, NH, S], BF16)
            kT = big.tile([128, S], BF16)
            v_nat = big.tile([128, NT, HD], BF16)

            # ---------------- norm + rope for one s-tile; q_src/k_src may be
            # PSUM (steady state) or SBUF (evacuated group 0)
            pending_tp = []

            def flush_tp():
                if pending_tp:
                    pending_tp.pop(0)()

            def emit_normrope(si, q_src, k_src, emit_vcopy, v_src):
                # q norm chain: ACT squares + accum
                scr = small.tile([128, 128], F32, tag="scr")
                ssq = small.tile([128, 8], F32, tag="ssq")
                for h in range(NH):
                    nc.scalar.activation(scr[:], q_src[:, h * 128:(h + 1) * 128],
                                         AF.Square, accum_out=ssq[:, h:h + 1])
                mn = small.tile([128, 8], F32, tag="mn")
                nc.vector.tensor_scalar(mn[:, 0:4], ssq[:, 0:4], 1.0 / HD, EPS,
                                        AluOpType.mult, AluOpType.add)
                rt = small.tile([128, 8], F32, tag="rt")
                nc.scalar.sqrt(rt[:, 0:4], mn[:, 0:4])
                rn = small.tile([128, 8], F32, tag="rn")
                nc.vector.reciprocal(rn[:, 0:4], rt[:, 0:4])
                qsc = small.tile([128, 4], F32, tag="qsc")
                nc.vector.tensor_tensor(qsc[:], rn[:, 0:4], gsm_bc[:], AluOpType.mult)

                qs = ropep.tile([128, 4, 128], F32, tag="qs")
                qsc_b = qsc[:, 0:4].unsqueeze(2).broadcast_to([128, 4, 128])
                q3 = q_src.rearrange("p (h d) -> p h d", h=4)
                nc.vector.tensor_tensor(qs[:], q3, qsc_b, AluOpType.mult)
                cos_b = cos2[:, si, :].unsqueeze(1).broadcast_to([128, 4, 128])
                tcs = ropep.tile([128, 4, 128], F32, tag="tcs")
                nc.vector.tensor_tensor(tcs[:], qs[:], cos_b, AluOpType.mult)
                tsn = ropep.tile([128, 4, 128], F32, tag="tsn")
                sinA = sin2[:, si, 0:64].unsqueeze(1).broadcast_to([128, 4, 64])
                sinB = sin2[:, si, 64:128].unsqueeze(1).broadcast_to([128, 4, 64])
                nc.vector.tensor_tensor(tsn[:, :, 0:64], qs[:, :, 64:128], sinA,
                                        AluOpType.mult)
                nc.vector.tensor_tensor(tsn[:, :, 64:128], qs[:, :, 0:64], sinB,
                                        AluOpType.mult)
                natq = ropep.tile([128, 4, 128], BF16, tag="natq", bufs=6)
                nc.gpsimd.tensor_tensor(natq[:], tcs[:], tsn[:], AluOpType.add)

                # k norm chain
                nc.scalar.activation(scr[:], k_src, AF.Square,
                                     accum_out=ssq[:, 4:5])
                nc.vector.tensor_scalar(mn[:, 4:5], ssq[:, 4:5], 1.0 / HD, EPS,
                                        AluOpType.mult, AluOpType.add)
                nc.scalar.sqrt(rt[:, 4:5], mn[:, 4:5])
                nc.vector.reciprocal(rn[:, 4:5], rt[:, 4:5])
                kcs = ropep.tile([128, 128], F32, tag="kcs")
                ksn = ropep.tile([128, 128], F32, tag="ksn")
                nc.vector.scalar_tensor_tensor(
                    kcs[:], k_src, rn[:, 4:5], cos2[:, si, :], AluOpType.mult,
                    AluOpType.mult)
                nc.vector.scalar_tensor_tensor(
                    ksn[:, 0:64], k_src[:, 64:128], rn[:, 4:5], sin2[:, si, 0:64],
                    AluOpType.mult, AluOpType.mult)
                nc.vector.scalar_tensor_tensor(
                    ksn[:, 64:128], k_src[:, 0:64], rn[:, 4:5], sin2[:, si, 64:128],
                    AluOpType.mult, AluOpType.mult)
                natk = ropep.tile([128, 128], BF16, tag="natk", bufs=6)
                nc.gpsimd.tensor_tensor(natk[:], kcs[:], ksn[:], AluOpType.add)

                if emit_vcopy:
                    nc.vector.tensor_copy(v_nat[:, si, :], v_src)

                def tp_(si=si, natq=natq, natk=natk):
                    for h in range(NH):
                        tp = psS.tile([128, 128], BF16, tag="S", name=f"tpq_{si}_{h}")
                        nc.tensor.transpose(tp[:], natq[:, h, :], ident[:])
                        nc.vector.tensor_copy(qT[:, h, si * 128:(si + 1) * 128], tp[:])
                    tp = psS.tile([128, 128], BF16, tag="S", name=f"tpk_{si}")
                    nc.tensor.transpose(tp[:], natk[:], ident[:])
                    nc.vector.tensor_copy(kT[:, si * 128:(si + 1) * 128], tp[:])
                pending_tp.append(tp_)

            # ---------------- phase 1a: group 0 ci-major, ACT-evacuated
            q_ps0 = [psA.tile([128, 512], F32, tag="A", name="qps0_0"),
                     psA.tile([128, 512], F32, tag="A", name="qps0_1"),
                     psS.tile([128, 512], F32, tag="S", name="qps0_2"),
                     psS.tile([128, 512], F32, tag="S", name="qps0_3")]
            kvp0 = [psB.tile([128, 512], F32, tag="B", name="kvp0_01"),
                    psB.tile([128, 512], F32, tag="B", name="kvp0_23")]
            for gi in range(4):
                for sl in range(4):
                    for a in range(4):
                        nc.tensor.matmul(
                            q_ps0[sl][:],
                            xsp[0][gi][:, a, sl * 128:(sl + 1) * 128],
                            wq_sb[gi][:, a, :],
                            start=(gi == 0 and a == 0), stop=(gi == 3 and a == 3),
                            skip_group_check=True)
                for sl in range(4):
                    kvp = kvp0[sl // 2]
                    koff = (sl % 2) * 256
                    for a in range(4):
                        # start=True zeroes the whole bank: only its first
                        # matmul sets it; odd region accumulates on zeros
                        nc.tensor.matmul(
                            kvp[:, koff:koff + 256],
                            xsp[0][gi][:, a, sl * 128:(sl + 1) * 128],
                            wkv_sb[gi][:, a, :],
                            start=(gi == 0 and a == 0 and sl % 2 == 0),
                            stop=(gi == 3 and a == 3 and sl % 2 == 1),
                            skip_group_check=True)
            for sl in range(4):
                kvp = kvp0[sl // 2]
                koff = (sl % 2) * 256
                q_sb = qsbp.tile([128, 512], F32, tag="qsb", name=f"qsb{sl}")
                nc.scalar.copy(q_sb[:], q_ps0[sl][:])
                k_sb = qsbp.tile([128, 128], F32, tag="ksb", name=f"ksb{sl}")
                nc.scalar.copy(k_sb[:], kvp[:, koff:koff + 128])
                nc.scalar.copy(v_nat[:, sl, :], kvp[:, koff + 128:koff + 256])
                emit_normrope(sl, q_sb[:], k_sb[:], False, None)

            # ---------------- phase 1b: si-major for si 4..15
            def emit_qkv(si):
                g, sl = si // 4, si % 4
                q_ps = psA.tile([128, 512], F32, tag="A", name=f"qps{si}")
                kv_ps = psB.tile([128, 256], F32, tag="B", name=f"kvps{si}")
                for i in range(NC_):
                    nc.tensor.matmul(
                        q_ps[:], xsp[g][i // 4][:, i % 4, sl * 128:(sl + 1) * 128],
                        wq_sb[i // 4][:, i % 4, :],
                        start=(i == 0), stop=(i == NC_ - 1))
                for i in range(NC_):
                    nc.tensor.matmul(
                        kv_ps[:], xsp[g][i // 4][:, i % 4, sl * 128:(sl + 1) * 128],
                        wkv_sb[i // 4][:, i % 4, :],
                        start=(i == 0), stop=(i == NC_ - 1))
                emit_normrope(si, q_ps[:], kv_ps[:, 0:128], True, kv_ps[:, 128:256])
                flush_tp()

            for si in range(4, NT):
                emit_qkv(si)

            # ---------------- phase 2: attention (no row-sum matmuls) + per-t
            # projection
            pending_rs = [None]
            pending_epi = [None]

            def flush_rs():
                if pending_rs[0] is not None:
                    pending_rs[0]()
                    pending_rs[0] = None

            def flush_epi():
                if pending_epi[0] is not None:
                    pending_epi[0]()
                    pending_epi[0] = None

            for t in range(T):
                yt_t = ytp.tile([128, NH, 512], BF16, tag="yt", name=f"yt{t}")
                nblk = 4 * t + 4
                for hp in (0, 2):
                    o_ps = {}
                    for h in (hp, hp + 1):
                        o_ps[h] = psA.tile([128, 512], F32, tag="A",
                                           name=f"o_ps_{t}_{h}")
                    acc = accp.tile([128, 1024], BF16, tag="acc",
                                    name=f"acc_{t}_{hp}")
                    prb_tiles = {}
                    pending_add = [None]

                    def emit_sc(j, t=t, hp=hp, acc=acc, prb_tiles=prb_tiles,
                                pending_add=pending_add):
                        off = j - 4 * t
                        q0 = max(off, 0) * 128
                        sc = psS.tile([128, 1024], F32, tag="S",
                                      name=f"sc_{t}_{hp}_{j}")
                        for u, h in enumerate((hp, hp + 1)):
                            nc.tensor.matmul(
                                sc[:, u * 512 + q0:(u + 1) * 512],
                                kT[:, j * 128:(j + 1) * 128],
                                qT[:, h, t * 512 + q0:(t + 1) * 512],
                                start=True, stop=True, skip_group_check=True)
                        prb = probsp.tile([128, 1024], BF16, tag="probs",
                                          name=f"prb_{t}_{hp}_{j}")
                        if off <= 0:
                            nc.scalar.activation(prb[:], sc[:], AF.Exp)
                        else:
                            for u in range(2):
                                nc.scalar.activation(
                                    prb[:, u * 512 + q0:(u + 1) * 512],
                                    sc[:, u * 512 + q0:(u + 1) * 512], AF.Exp)
                        if off >= 0:
                            pr2 = prb[:].rearrange("p (u c) -> p u c", u=2)
                            tri_b = tri[:].unsqueeze(1).broadcast_to([128, 2, 128])
                            nc.vector.tensor_tensor(pr2[:, :, q0:q0 + 128],
                                                    pr2[:, :, q0:q0 + 128],
                                                    tri_b, AluOpType.mult)
                        # acc add delayed one block so the next diag mask isn't
                        # queued behind it on DVE
                        if pending_add[0] is not None:
                            pending_add[0]()

                        def mk_add(j=j, off=off, q0=q0, prb=prb, acc=acc):
                            def f():
                                if j == 0:
                                    nc.vector.tensor_copy(acc[:], prb[:])
                                elif off <= 0:
                                    nc.vector.tensor_tensor(
                                        acc[:], acc[:], prb[:], AluOpType.add)
                                else:
                                    ac2 = acc[:].rearrange("p (u c) -> p u c", u=2)
                                    pr2 = prb[:].rearrange("p (u c) -> p u c", u=2)
                                    nc.vector.tensor_tensor(
                                        ac2[:, :, q0:512], ac2[:, :, q0:512],
                                        pr2[:, :, q0:512], AluOpType.add)
                            return f
                        pending_add[0] = mk_add()
                        prb_tiles[j] = prb

                    def emit_pv(j, t=t, hp=hp, o_ps=o_ps, prb_tiles=prb_tiles,
                                nblk=nblk):
                        off = j - 4 * t
                        q0 = max(off, 0) * 128
                        prb = prb_tiles.pop(j)
                        first = (j == 0)
                        last = (j == nblk - 1)
                        for u, h in enumerate((hp, hp + 1)):
                            nc.tensor.matmul(
                                o_ps[h][:, q0:512], v_nat[:, j, :],
                                prb[:, u * 512 + q0:(u + 1) * 512],
                                start=first, stop=last, skip_group_check=True)

                    depth = 3
                    for j in range(depth):
                        emit_sc(j)
                        if j == 1:
                            flush_rs()
                    for j in range(depth, nblk):
                        emit_sc(j)
                        emit_pv(j - depth)
                        if j == depth:
                            flush_epi()
                    for j in range(nblk - depth, nblk):
                        emit_pv(j)
                    if pending_add[0] is not None:
                        pending_add[0]()
                        pending_add[0] = None
                    flush_tp()   # drain leftover transpose groups (si 12..15)

                    rs_cell = {}

                    def make_rs(hp=hp, acc=acc, rs_cell=rs_cell, t=t):
                        def f():
                            for u, h in enumerate((hp, hp + 1)):
                                r = psB.tile([128, 512], F32, tag="B",
                                             name=f"rs_{t}_{h}")
                                nc.tensor.matmul(r[:], onesq[:],
                                                 acc[:, u * 512:(u + 1) * 512],
                                                 start=True, stop=True,
                                                 skip_group_check=True)
                                rs_cell[h] = r
                        return f

                    def make_epi(hp=hp, o_ps=o_ps, rs_cell=rs_cell, yt_t=yt_t):
                        def f():
                            rcp0 = normp.tile([128, 512], F32, tag="rcp")
                            rcp1 = normp.tile([128, 512], F32, tag="rcp")
                            nc.vector.reciprocal_approx_fast(rcp0[:], rs_cell[hp][:])
                            nc.vector.reciprocal_approx_fast(rcp1[:],
                                                             rs_cell[hp + 1][:])
                            nc.vector.tensor_tensor(
                                yt_t[:, hp, :], o_ps[hp][:], rcp0[:],
                                AluOpType.mult)
                            nc.vector.tensor_tensor(
                                yt_t[:, hp + 1, :], o_ps[hp + 1][:], rcp1[:],
                                AluOpType.mult)
                        return f

                    pending_rs[0] = make_rs()
                    pending_epi[0] = make_epi()

                # flush the unit's rs+epilogue before proj so the DVE chain
                # overlaps the first pj matmuls
                flush_rs()
                flush_epi()

                # ---------------- phase 3 (per t): projection
                for si in range(4 * t, 4 * t + 4):
                    sl = si - 4 * t
                    for dtp in range(2):
                        if (2 * sl + dtp) % 2 == 0:
                            pj0 = psB.tile([128, 512], F32, tag="B",
                                           name=f"pj_{si}_{dtp}_0")
                            pj1 = psB.tile([128, 512], F32, tag="B",
                                           name=f"pj_{si}_{dtp}_1")
                        else:
                            pjS = psS.tile([128, 1024], F32, tag="S",
                                           name=f"pj_{si}_{dtp}_s")
                            pj0 = pjS[:, 0:512]
                            pj1 = pjS[:, 512:1024]
                        for h in range(NH):
                            lhs = yt_t[:, h, sl * 128:(sl + 1) * 128]
                            nc.tensor.matmul(pj0[:], lhs, wp[2 * dtp][:, h, :],
                                             start=(h == 0), stop=(h == NH - 1),
                                             skip_group_check=True)
                            nc.tensor.matmul(pj1[:], lhs, wp[2 * dtp + 1][:, h, :],
                                             start=(h == 0), stop=(h == NH - 1),
                                             skip_group_check=True)
                        ev = outsb.tile([128, 1024], BF16, tag="ev",
                                        name=f"ev_{si}_{dtp}")
                        nc.scalar.copy(ev[:, 0:512], pj0[:])
                        nc.vector.tensor_copy(ev[:, 512:1024], pj1[:])
                        yq = (2 * si + dtp) % 3
                        if yq == 0:
                            nc.sync.dma_start(Y[si, dtp], ev[:])
                        elif yq == 1:
                            nc.gpsimd.dma_start(Y[si, dtp], ev[:])
                        else:
                            nc.scalar.dma_start(Y[si, dtp], ev[:])

    nc.compile()
    return nc


def _host_inputs(x, Wq, Wk, Wv, Wproj, q_gain):
    import ml_dtypes
    bf16 = ml_dtypes.bfloat16

    x = np.asarray(x, dtype=np.float32)
    Wq = np.asarray(Wq, dtype=np.float32)
    Wk = np.asarray(Wk, dtype=np.float32)
    Wv = np.asarray(Wv, dtype=np.float32)
    Wproj = np.asarray(Wproj, dtype=np.float32)
    q_gain = np.asarray(q_gain, dtype=np.float32)

    inv = (1.0 / ROPE_BASE ** (np.arange(0, HD, 2, dtype=np.float32) / HD)).astype(np.float32)
    ang = np.outer(np.arange(S, dtype=np.float32), inv)
    cos = np.cos(ang).astype(np.float32)
    sin = np.sin(ang).astype(np.float32)
    cos2 = np.concatenate([cos, cos], 1).reshape(NT, 128, HD).transpose(1, 0, 2)
    sin2 = np.concatenate([sin, -sin], 1).reshape(NT, 128, HD).transpose(1, 0, 2)
    cs = np.ascontiguousarray(np.stack([cos2, sin2])).astype(bf16)  # [2,128,NT,HD]

    qq = np.arange(128)[None, :]
    kk = np.arange(128)[:, None]
    const3 = np.ascontiguousarray(np.stack(
        [np.eye(128, dtype=np.float32),
         np.ones((128, 128), dtype=np.float32),
         (kk <= qq).astype(np.float32)], axis=1)).astype(bf16)  # [128,3,128]

    xTb = [np.ascontiguousarray(
        x[b].T.astype(bf16).reshape(4, 4, 128, 4, 512).transpose(3, 0, 2, 1, 4))
        for b in range(B)]

    in_maps = []
    for cid in range(8):
        b, g = cid // 4, cid % 4
        wq = Wq[g * 512:(g + 1) * 512, :].T            # [D, 512]
        wk = Wk[g * 128:(g + 1) * 128, :].T            # [D, 128]
        wv = Wv[g * 128:(g + 1) * 128, :].T
        wkv = np.concatenate([wk, wv], 1)              # [D, 256]
        wp = Wproj[:, g * 512:(g + 1) * 512].T         # [512, D] (c_local, dout)
        in_maps.append({
            "XT": xTb[b],
            "WQ": np.ascontiguousarray(
                wq.reshape(4, 4, 128, 512).transpose(0, 2, 1, 3)).astype(bf16),
            "WKV": np.ascontiguousarray(
                wkv.reshape(4, 4, 128, 256).transpose(0, 2, 1, 3)).astype(bf16),
            "WP": np.ascontiguousarray(
                wp.reshape(NH, 128, 4, 512).transpose(2, 1, 0, 3)).astype(bf16),
            "CS": cs,
            "GSM": (q_gain[g * 4:(g + 1) * 4] * SM).reshape(1, NH).astype(np.float32),
            "CONST3": const3,
        })
    return in_maps


def _get_prog():
    global _PROG
    if _PROG is None:
        _PROG = _build_program()
    return _PROG


def kernel(x, Wq, Wk, Wv, Wproj, q_gain, _trace=False, _tmpdir=None):
    from concourse.bass_utils import run_bass_kernel_spmd
    nc = _get_prog()
    in_maps = _host_inputs(x, Wq, Wk, Wv, Wproj, q_gain)
    kwargs = {}
    if _tmpdir is not None:
        os.makedirs(_tmpdir, exist_ok=True)
        kwargs["tmpdir"] = _tmpdir
    res = run_bass_kernel_spmd(nc, in_maps, list(range(8)), trace=_trace, **kwargs)
    y = np.empty((B, S, D), dtype=np.float32)
    for b in range(B):
        acc = res.results[4 * b]["Y"].astype(np.float32)
        for g in range(1, 4):
            acc = acc + res.results[4 * b + g]["Y"].astype(np.float32)
        y[b] = acc.transpose(0, 2, 1, 3).reshape(S, D)
    if _trace:
        kernel._last_result = res
    return y


# revision 23
# speedup vs baseline: 1.1730x; 1.1730x over previous
"""Causal GQA self-attention (B=2, S=2048, D=2048, H=16, KV=4) on 8 TRN2 cores.

Sharding: core = (b, g) with b = batch (2) x g = kv-head group (4).
Each core computes 4 q-heads / 1 kv-head for one batch and a partial
projection output [S, D] in bf16; host sums the 4 group partials per batch.

v4 = v2 (3-phase, 303us) + targeted wins:
  - startup: group-0 QKV consumed ci-major in DMA arrival order (each
    weight/x chunk feeds 4 s-tiles), q_ps for si2/3 borrow the score psum
    slots, retuned 3-queue DMA order. Cuts ~15us of PE idle at startup.
  - row-sum matmuls removed (~26us of PE streaming): probs accumulated
    per-unit on DVE into a bf16 sbuf tile (diag blocks via one strided op),
    one small ones-matmul pair per unit; acc adds delayed one block so diag
    masks aren't queued behind them on DVE.
  - projection psum pairs alternate between B banks and an S slot
    (deeper rotation, no stall on the ev-copy latency); ev DMA alternates
    sync/gpsimd queues; 6 ev buffers.
"""
import os
import sys

if '/opt/trn_rl_repo' not in sys.path:
    sys.path.insert(0, '/opt/trn_rl_repo')

import numpy as np

B, S, D = 2, 2048, 2048
NH_TOT, NKV_TOT, HD = 16, 4, 128
NH = 4                 # q heads per core
NT = S // 128          # 16 s-tiles
NC_ = D // 128         # 16 c-tiles
T = 4                  # q-slices of 512
SM = 1.0 / np.sqrt(HD)
EPS = float(np.finfo(np.float32).eps)
ROPE_BASE = 10000.0

_PROG = None


def _build_program():
    import concourse.bass as bass
    import concourse.mybir as mybir
    import concourse.tile as tile
    from concourse import bacc
    from concourse.alu_op_type import AluOpType

    F32 = mybir.dt.float32
    BF16 = mybir.dt.bfloat16
    AF = mybir.ActivationFunctionType

    nc = bacc.Bacc("TRN2", target_bir_lowering=False, debug=False)

    XT = nc.dram_tensor("XT", [4, 4, 128, 4, 512], BF16, kind="ExternalInput")
    WQ = nc.dram_tensor("WQ", [4, 128, 4, 512], BF16, kind="ExternalInput")   # [c4, c_p, a, dq]
    WKV = nc.dram_tensor("WKV", [4, 128, 4, 256], BF16, kind="ExternalInput")
    WP = nc.dram_tensor("WP", [4, 128, NH, 512], BF16, kind="ExternalInput")  # [dq, c_in_head, h, dout]
    CS = nc.dram_tensor("CS", [2, 128, NT, HD], BF16, kind="ExternalInput")   # cos;sin
    GSM = nc.dram_tensor("GSM", [1, NH], F32, kind="ExternalInput")           # gain*sm per head
    CONST3 = nc.dram_tensor("CONST3", [128, 3, 128], BF16, kind="ExternalInput")  # ident|onesq|tri
    Y = nc.dram_tensor("Y", [NT, 2, 128, 1024], BF16, kind="ExternalOutput")

    with tile.TileContext(nc) as tc:
        with (
            tc.tile_pool(name="const", bufs=1) as const,
            tc.tile_pool(name="w", bufs=4) as wpool,
            tc.tile_pool(name="stream", bufs=3) as stream,
            tc.tile_pool(name="small", bufs=3) as small,
            tc.tile_pool(name="norm", bufs=4) as normp,
            tc.tile_pool(name="rope", bufs=2) as ropep,
            tc.tile_pool(name="qsb", bufs=4) as qsbp,
            tc.tile_pool(name="big", bufs=1) as big,
            tc.tile_pool(name="yt", bufs=2) as ytp,
            tc.tile_pool(name="probs", bufs=4) as probsp,
            tc.tile_pool(name="accp", bufs=2) as accp,
            tc.tile_pool(name="outsb", bufs=6) as outsb,
            tc.tile_pool(name="psA", bufs=2, space="PSUM") as psA,
            tc.tile_pool(name="psB", bufs=2, space="PSUM") as psB,
            tc.tile_pool(name="psS", bufs=2, space="PSUM") as psS,
        ):
            wq_sb = [wpool.tile([128, 4, 512], BF16, tag="wq", name=f"wq{i}")
                     for i in range(4)]
            wkv_sb = [wpool.tile([128, 4, 256], BF16, tag="wkv", name=f"wkv{i}")
                      for i in range(4)]
            wp = [wpool.tile([128, NH, 512], BF16, tag="wp", name=f"wp{i}")
                  for i in range(4)]
            xsp = [[stream.tile([128, 4, 512], BF16, tag="xs", bufs=12,
                                name=f"xs_{s4}_{c4}")
                    for c4 in range(4)] for s4 in range(4)]

            gsm = const.tile([1, NH], F32)
            const3 = const.tile([128, 3, 128], BF16)
            cs = const.tile([128, 2, NT, HD], BF16)

            # scalar queue: gsm, WQ0, WKV0, WQ2, WKV2, cs, const3, WP
            nc.scalar.dma_start(gsm[:], GSM[:])
            nc.scalar.dma_start(wq_sb[0][:], WQ[0])
            nc.scalar.dma_start(wkv_sb[0][:], WKV[0])
            nc.scalar.dma_start(wq_sb[2][:], WQ[2])
            nc.scalar.dma_start(wkv_sb[2][:], WKV[2])
            nc.scalar.dma_start(cs[:, 0], CS[0])
            nc.scalar.dma_start(cs[:, 1], CS[1])
            nc.scalar.dma_start(const3[:], CONST3[:])
            for dq in range(4):
                nc.scalar.dma_start(wp[dq][:], WP[dq])

            # sync queue: XT00, XT01, WQ3, WKV3, XT1 c0/c1, then Y duty
            nc.sync.dma_start(xsp[0][0][:], XT[0, 0])
            nc.sync.dma_start(xsp[0][1][:], XT[0, 1])
            nc.sync.dma_start(wq_sb[3][:], WQ[3])
            nc.sync.dma_start(wkv_sb[3][:], WKV[3])
            nc.sync.dma_start(xsp[1][0][:], XT[1, 0])
            nc.sync.dma_start(xsp[1][1][:], XT[1, 1])
            nc.sync.dma_start(xsp[2][2][:], XT[2, 2])
            nc.sync.dma_start(xsp[2][3][:], XT[2, 3])
            nc.sync.dma_start(xsp[3][2][:], XT[3, 2])
            nc.sync.dma_start(xsp[3][3][:], XT[3, 3])

            # gpsimd queue: WQ1, WKV1, XT02, XT03, XT1 c2/c3, XT2, XT3, Y duty
            nc.gpsimd.dma_start(wq_sb[1][:], WQ[1])
            nc.gpsimd.dma_start(wkv_sb[1][:], WKV[1])
            nc.gpsimd.dma_start(xsp[0][2][:], XT[0, 2])
            nc.gpsimd.dma_start(xsp[0][3][:], XT[0, 3])
            nc.gpsimd.dma_start(xsp[1][2][:], XT[1, 2])
            nc.gpsimd.dma_start(xsp[1][3][:], XT[1, 3])
            for c4 in range(2):
                nc.gpsimd.dma_start(xsp[2][c4][:], XT[2, c4])
            for c4 in range(2):
                nc.gpsimd.dma_start(xsp[3][c4][:], XT[3, c4])

            ident = const3[:, 0, :]
            onesq = const3[:, 1, :]
            tri = const3[:, 2, :]
            cos2 = cs[:, 0]
            sin2 = cs[:, 1]
            gsm_bc = const.tile([128, NH], F32)
            nc.gpsimd.partition_broadcast(gsm_bc[:], gsm[:])

            qT = big.tile([128, NH, S], BF16)
            kT = big.tile([128, S], BF16)
            v_nat = big.tile([128, NT, HD], BF16)

            # ---------------- norm + rope for one s-tile; q_src/k_src may be
            # PSUM (steady state) or SBUF (evacuated group 0)
            pending_tp = []

            def flush_tp():
                if pending_tp:
                    pending_tp.pop(0)()

            def emit_normrope(si, q_src, k_src, emit_vcopy, v_src):
                # q norm chain: ACT squares + accum
                scr = small.tile([128, 128], F32, tag="scr")
                ssq = small.tile([128, 8], F32, tag="ssq")
                for h in range(NH):
                    nc.scalar.activation(scr[:], q_src[:, h * 128:(h + 1) * 128],
                                         AF.Square, accum_out=ssq[:, h:h + 1])
                mn = small.tile([128, 8], F32, tag="mn")
                nc.vector.tensor_scalar(mn[:, 0:4], ssq[:, 0:4], 1.0 / HD, EPS,
                                        AluOpType.mult, AluOpType.add)
                rt = small.tile([128, 8], F32, tag="rt")
                nc.scalar.sqrt(rt[:, 0:4], mn[:, 0:4])
                rn = small.tile([128, 8], F32, tag="rn")
                nc.vector.reciprocal(rn[:, 0:4], rt[:, 0:4])
                qsc = small.tile([128, 4], F32, tag="qsc")
                nc.vector.tensor_tensor(qsc[:], rn[:, 0:4], gsm_bc[:], AluOpType.mult)

                qs = ropep.tile([128, 4, 128], F32, tag="qs")
                qsc_b = qsc[:, 0:4].unsqueeze(2).broadcast_to([128, 4, 128])
                q3 = q_src.rearrange("p (h d) -> p h d", h=4)
                nc.vector.tensor_tensor(qs[:], q3, qsc_b, AluOpType.mult)
                cos_b = cos2[:, si, :].unsqueeze(1).broadcast_to([128, 4, 128])
                tcs = ropep.tile([128, 4, 128], F32, tag="tcs")
                nc.vector.tensor_tensor(tcs[:], qs[:], cos_b, AluOpType.mult)
                tsn = ropep.tile([128, 4, 128], F32, tag="tsn")
                sinA = sin2[:, si, 0:64].unsqueeze(1).broadcast_to([128, 4, 64])
                sinB = sin2[:, si, 64:128].unsqueeze(1).broadcast_to([128, 4, 64])
                nc.vector.tensor_tensor(tsn[:, :, 0:64], qs[:, :, 64:128], sinA,
                                        AluOpType.mult)
                nc.vector.tensor_tensor(tsn[:, :, 64:128], qs[:, :, 0:64], sinB,
                                        AluOpType.mult)
                natq = ropep.tile([128, 4, 128], BF16, tag="natq", bufs=6)
                nc.gpsimd.tensor_tensor(natq[:], tcs[:], tsn[:], AluOpType.add)

                # k norm chain
                nc.scalar.activation(scr[:], k_src, AF.Square,
                                     accum_out=ssq[:, 4:5])
                nc.vector.tensor_scalar(mn[:, 4:5], ssq[:, 4:5], 1.0 / HD, EPS,
                                        AluOpType.mult, AluOpType.add)
                nc.scalar.sqrt(rt[:, 4:5], mn[:, 4:5])
                nc.vector.reciprocal(rn[:, 4:5], rt[:, 4:5])
                kcs = ropep.tile([128, 128], F32, tag="kcs")
                ksn = ropep.tile([128, 128], F32, tag="ksn")
                nc.vector.scalar_tensor_tensor(
                    kcs[:], k_src, rn[:, 4:5], cos2[:, si, :], AluOpType.mult,
                    AluOpType.mult)
                nc.vector.scalar_tensor_tensor(
                    ksn[:, 0:64], k_src[:, 64:128], rn[:, 4:5], sin2[:, si, 0:64],
                    AluOpType.mult, AluOpType.mult)
                nc.vector.scalar_tensor_tensor(
                    ksn[:, 64:128], k_src[:, 0:64], rn[:, 4:5], sin2[:, si, 64:128],
                    AluOpType.mult, AluOpType.mult)
                natk = ropep.tile([128, 128], BF16, tag="natk", bufs=6)
                nc.gpsimd.tensor_tensor(natk[:], kcs[:], ksn[:], AluOpType.add)

                if emit_vcopy:
                    nc.vector.tensor_copy(v_nat[:, si, :], v_src)

                def tp_(si=si, natq=natq, natk=natk):
                    for h in range(NH):
                        tp = psS.tile([128, 128], BF16, tag="S", name=f"tpq_{si}_{h}")
                        nc.tensor.transpose(tp[:], natq[:, h, :], ident[:])
                        nc.vector.tensor_copy(qT[:, h, si * 128:(si + 1) * 128], tp[:])
                    tp = psS.tile([128, 128], BF16, tag="S", name=f"tpk_{si}")
                    nc.tensor.transpose(tp[:], natk[:], ident[:])
                    nc.vector.tensor_copy(kT[:, si * 128:(si + 1) * 128], tp[:])
                pending_tp.append(tp_)

            # ---------------- phase 1a: group 0 ci-major, ACT-evacuated
            q_ps0 = [psA.tile([128, 512], F32, tag="A", name="qps0_0"),
                     psA.tile([128, 512], F32, tag="A", name="qps0_1"),
                     psS.tile([128, 512], F32, tag="S", name="qps0_2"),
                     psS.tile([128, 512], F32, tag="S", name="qps0_3")]
            kvp0 = [psB.tile([128, 512], F32, tag="B", name="kvp0_01"),
                    psB.tile([128, 512], F32, tag="B", name="kvp0_23")]
            for gi in range(4):
                for sl in range(4):
                    for a in range(4):
                        nc.tensor.matmul(
                            q_ps0[sl][:],
                            xsp[0][gi][:, a, sl * 128:(sl + 1) * 128],
                            wq_sb[gi][:, a, :],
                            start=(gi == 0 and a == 0), stop=(gi == 3 and a == 3),
                            skip_group_check=True)
                for sl in range(4):
                    kvp = kvp0[sl // 2]
                    koff = (sl % 2) * 256
                    for a in range(4):
                        # start=True zeroes the whole bank: only its first
                        # matmul sets it; odd region accumulates on zeros
                        nc.tensor.matmul(
                            kvp[:, koff:koff + 256],
                            xsp[0][gi][:, a, sl * 128:(sl + 1) * 128],
                            wkv_sb[gi][:, a, :],
                            start=(gi == 0 and a == 0 and sl % 2 == 0),
                            stop=(gi == 3 and a == 3 and sl % 2 == 1),
                            skip_group_check=True)
            for sl in range(4):
                kvp = kvp0[sl // 2]
                koff = (sl % 2) * 256
                q_sb = qsbp.tile([128, 512], F32, tag="qsb", name=f"qsb{sl}")
                nc.scalar.copy(q_sb[:], q_ps0[sl][:])
                k_sb = qsbp.tile([128, 128], F32, tag="ksb", name=f"ksb{sl}")
                nc.scalar.copy(k_sb[:], kvp[:, koff:koff + 128])
                nc.scalar.copy(v_nat[:, sl, :], kvp[:, koff + 128:koff + 256])
                emit_normrope(sl, q_sb[:], k_sb[:], False, None)

            # ---------------- phase 1b: si-major for si 4..15
            def emit_qkv(si):
                g, sl = si // 4, si % 4
                q_ps = psA.tile([128, 512], F32, tag="A", name=f"qps{si}")
                kv_ps = psB.tile([128, 256], F32, tag="B", name=f"kvps{si}")
                for i in range(NC_):
                    nc.tensor.matmul(
                        q_ps[:], xsp[g][i // 4][:, i % 4, sl * 128:(sl + 1) * 128],
                        wq_sb[i // 4][:, i % 4, :],
                        start=(i == 0), stop=(i == NC_ - 1))
                for i in range(NC_):
                    nc.tensor.matmul(
                        kv_ps[:], xsp[g][i // 4][:, i % 4, sl * 128:(sl + 1) * 128],
                        wkv_sb[i // 4][:, i % 4, :],
                        start=(i == 0), stop=(i == NC_ - 1))
                emit_normrope(si, q_ps[:], kv_ps[:, 0:128], True, kv_ps[:, 128:256])
                flush_tp()

            for si in range(4, NT):
                emit_qkv(si)

            # ---------------- phase 2: attention (no row-sum matmuls) + per-t
            # projection
            pending_rs = [None]
            pending_epi = [None]

            def flush_rs():
                if pending_rs[0] is not None:
                    pending_rs[0]()
                    pending_rs[0] = None

            def flush_epi():
                if pending_epi[0] is not None:
                    pending_epi[0]()
                    pending_epi[0] = None

            for t in range(T):
                yt_t = ytp.tile([128, NH, 512], BF16, tag="yt", name=f"yt{t}")
                nblk = 4 * t + 4
                for hp in (0, 2):
                    o_ps = {}
                    for h in (hp, hp + 1):
                        o_ps[h] = psA.tile([128, 512], F32, tag="A",
                                           name=f"o_ps_{t}_{h}")
                    acc = accp.tile([128, 1024], BF16, tag="acc",
                                    name=f"acc_{t}_{hp}")
                    prb_tiles = {}
                    pending_add = [None]

                    def emit_sc(j, t=t, hp=hp, acc=acc, prb_tiles=prb_tiles,
                                pending_add=pending_add):
                        off = j - 4 * t
                        q0 = max(off, 0) * 128
                        sc = psS.tile([128, 1024], F32, tag="S",
                                      name=f"sc_{t}_{hp}_{j}")
                        for u, h in enumerate((hp, hp + 1)):
                            nc.tensor.matmul(
                                sc[:, u * 512 + q0:(u + 1) * 512],
                                kT[:, j * 128:(j + 1) * 128],
                                qT[:, h, t * 512 + q0:(t + 1) * 512],
                                start=True, stop=True, skip_group_check=True)
                        prb = probsp.tile([128, 1024], BF16, tag="probs",
                                          name=f"prb_{t}_{hp}_{j}")
                        if off <= 0:
                            nc.scalar.activation(prb[:], sc[:], AF.Exp)
                        else:
                            for u in range(2):
                                nc.scalar.activation(
                                    prb[:, u * 512 + q0:(u + 1) * 512],
                                    sc[:, u * 512 + q0:(u + 1) * 512], AF.Exp)
                        if off >= 0:
                            pr2 = prb[:].rearrange("p (u c) -> p u c", u=2)
                            tri_b = tri[:].unsqueeze(1).broadcast_to([128, 2, 128])
                            nc.vector.tensor_tensor(pr2[:, :, q0:q0 + 128],
                                                    pr2[:, :, q0:q0 + 128],
                                                    tri_b, AluOpType.mult)
                        # acc add delayed one block so the next diag mask isn't
                        # queued behind it on DVE
                        if pending_add[0] is not None:
                            pending_add[0]()

                        def mk_add(j=j, off=off, q0=q0, prb=prb, acc=acc):
                            def f():
                                if j == 0:
                                    nc.vector.tensor_copy(acc[:], prb[:])
                                elif off <= 0:
                                    nc.vector.tensor_tensor(
                                        acc[:], acc[:], prb[:], AluOpType.add)
                                else:
                                    ac2 = acc[:].rearrange("p (u c) -> p u c", u=2)
                                    pr2 = prb[:].rearrange("p (u c) -> p u c", u=2)
                                    nc.vector.tensor_tensor(
                                        ac2[:, :, q0:512], ac2[:, :, q0:512],
                                        pr2[:, :, q0:512], AluOpType.add)
                            return f
                        pending_add[0] = mk_add()
                        prb_tiles[j] = prb

                    def emit_pv(j, t=t, hp=hp, o_ps=o_ps, prb_tiles=prb_tiles,
                                nblk=nblk):
                        off = j - 4 * t
                        q0 = max(off, 0) * 128
                        prb = prb_tiles.pop(j)
                        first = (j == 0)
                        last = (j == nblk - 1)
                        for u, h in enumerate((hp, hp + 1)):
                            nc.tensor.matmul(
                                o_ps[h][:, q0:512], v_nat[:, j, :],
                                prb[:, u * 512 + q0:(u + 1) * 512],
                                start=first, stop=last, skip_group_check=True)

                    depth = 3
                    for j in range(depth):
                        emit_sc(j)
                        if j == 1:
                            flush_rs()
                    for j in range(depth, nblk):
                        emit_sc(j)
                        emit_pv(j - depth)
                        if j == depth:
                            flush_epi()
                    for j in range(nblk - depth, nblk):
                        emit_pv(j)
                    if pending_add[0] is not None:
                        pending_add[0]()
                        pending_add[0] = None
                    flush_tp()   # drain leftover transpose groups (si 12..15)

                    rs_cell = {}

                    def make_rs(hp=hp, acc=acc, rs_cell=rs_cell, t=t):
                        def f():
                            for u, h in enumerate((hp, hp + 1)):
                                r = psB.tile([128, 512], F32, tag="B",
                                             name=f"rs_{t}_{h}")
                                nc.tensor.matmul(r[:], onesq[:],
                                                 acc[:, u * 512:(u + 1) * 512],
                                                 start=True, stop=True,
                                                 skip_group_check=True)
                                rs_cell[h] = r
                        return f

                    def make_epi(hp=hp, o_ps=o_ps, rs_cell=rs_cell, yt_t=yt_t):
                        def f():
                            rcp0 = normp.tile([128, 512], F32, tag="rcp")
                            rcp1 = normp.tile([128, 512], F32, tag="rcp")
                            nc.vector.reciprocal_approx_fast(rcp0[:], rs_cell[hp][:])
                            nc.vector.reciprocal_approx_fast(rcp1[:],
                                                             rs_cell[hp + 1][:])
                            nc.vector.tensor_tensor(
                                yt_t[:, hp, :], o_ps[hp][:], rcp0[:],
                                AluOpType.mult)
                            nc.vector.tensor_tensor(
                                yt_t[:, hp + 1, :], o_ps[hp + 1][:], rcp1[:],
                                AluOpType.mult)
                        return f

                    pending_rs[0] = make_rs()
                    pending_epi[0] = make_epi()

                # flush the unit's rs+epilogue before proj so the DVE chain
                # overlaps the first pj matmuls
                flush_rs()
                flush_epi()

                # ---------------- phase 3 (per t): projection
                for si in range(4 * t, 4 * t + 4):
                    sl = si - 4 * t
                    for dtp in range(2):
                        if (2 * sl + dtp) % 2 == 0:
                            pj0 = psB.tile([128, 512], F32, tag="B",
                                           name=f"pj_{si}_{dtp}_0")
                            pj1 = psB.tile([128, 512], F32, tag="B",
                                           name=f"pj_{si}_{dtp}_1")
                        else:
                            pjS = psS.tile([128, 1024], F32, tag="S",
                                           name=f"pj_{si}_{dtp}_s")
                            pj0 = pjS[:, 0:512]
                            pj1 = pjS[:, 512:1024]
                        for h in range(NH):
                            lhs = yt_t[:, h, sl * 128:(sl + 1) * 128]
                            nc.tensor.matmul(pj0[:], lhs, wp[2 * dtp][:, h, :],
                                             start=(h == 0), stop=(h == NH - 1),
                                             skip_group_check=True)
                            nc.tensor.matmul(pj1[:], lhs, wp[2 * dtp + 1][:, h, :],
                                             start=(h == 0), stop=(h == NH - 1),
                                             skip_group_check=True)
                        ev = outsb.tile([128, 1024], BF16, tag="ev",
                                        name=f"ev_{si}_{dtp}")
                        nc.scalar.copy(ev[:, 0:512], pj0[:])
                        nc.vector.tensor_copy(ev[:, 512:1024], pj1[:])
                        if dtp == 0:
                            nc.sync.dma_start(Y[si, dtp], ev[:])
                        else:
                            nc.gpsimd.dma_start(Y[si, dtp], ev[:])

    nc.compile()
    return nc


def _host_inputs(x, Wq, Wk, Wv, Wproj, q_gain):
    import ml_dtypes
    bf16 = ml_dtypes.bfloat16

    x = np.asarray(x, dtype=np.float32)
    Wq = np.asarray(Wq, dtype=np.float32)
    Wk = np.asarray(Wk, dtype=np.float32)
    Wv = np.asarray(Wv, dtype=np.float32)
    Wproj = np.asarray(Wproj, dtype=np.float32)
    q_gain = np.asarray(q_gain, dtype=np.float32)

    inv = (1.0 / ROPE_BASE ** (np.arange(0, HD, 2, dtype=np.float32) / HD)).astype(np.float32)
    ang = np.outer(np.arange(S, dtype=np.float32), inv)
    cos = np.cos(ang).astype(np.float32)
    sin = np.sin(ang).astype(np.float32)
    cos2 = np.concatenate([cos, cos], 1).reshape(NT, 128, HD).transpose(1, 0, 2)
    sin2 = np.concatenate([sin, -sin], 1).reshape(NT, 128, HD).transpose(1, 0, 2)
    cs = np.ascontiguousarray(np.stack([cos2, sin2])).astype(bf16)  # [2,128,NT,HD]

    qq = np.arange(128)[None, :]
    kk = np.arange(128)[:, None]
    const3 = np.ascontiguousarray(np.stack(
        [np.eye(128, dtype=np.float32),
         np.ones((128, 128), dtype=np.float32),
         (kk <= qq).astype(np.float32)], axis=1)).astype(bf16)  # [128,3,128]

    xTb = [np.ascontiguousarray(
        x[b].T.astype(bf16).reshape(4, 4, 128, 4, 512).transpose(3, 0, 2, 1, 4))
        for b in range(B)]

    in_maps = []
    for cid in range(8):
        b, g = cid // 4, cid % 4
        wq = Wq[g * 512:(g + 1) * 512, :].T            # [D, 512]
        wk = Wk[g * 128:(g + 1) * 128, :].T            # [D, 128]
        wv = Wv[g * 128:(g + 1) * 128, :].T
        wkv = np.concatenate([wk, wv], 1)              # [D, 256]
        wp = Wproj[:, g * 512:(g + 1) * 512].T         # [512, D] (c_local, dout)
        in_maps.append({
            "XT": xTb[b],
            "WQ": np.ascontiguousarray(
                wq.reshape(4, 4, 128, 512).transpose(0, 2, 1, 3)).astype(bf16),
            "WKV": np.ascontiguousarray(
                wkv.reshape(4, 4, 128, 256).transpose(0, 2, 1, 3)).astype(bf16),
            "WP": np.ascontiguousarray(
                wp.reshape(NH, 128, 4, 512).transpose(2, 1, 0, 3)).astype(bf16),
            "CS": cs,
            "GSM": (q_gain[g * 4:(g + 1) * 4] * SM).reshape(1, NH).astype(np.float32),
            "CONST3": const3,
        })
    return in_maps


def _get_prog():
    global _PROG
    if _PROG is None:
        _PROG = _build_program()
    return _PROG


def kernel(x, Wq, Wk, Wv, Wproj, q_gain, _trace=False, _tmpdir=None):
    from concourse.bass_utils import run_bass_kernel_spmd
    nc = _get_prog()
    in_maps = _host_inputs(x, Wq, Wk, Wv, Wproj, q_gain)
    kwargs = {}
    if _tmpdir is not None:
        os.makedirs(_tmpdir, exist_ok=True)
        kwargs["tmpdir"] = _tmpdir
    res = run_bass_kernel_spmd(nc, in_maps, list(range(8)), trace=_trace, **kwargs)
    y = np.empty((B, S, D), dtype=np.float32)
    for b in range(B):
        acc = res.results[4 * b]["Y"].astype(np.float32)
        for g in range(1, 4):
            acc = acc + res.results[4 * b + g]["Y"].astype(np.float32)
        y[b] = acc.transpose(0, 2, 1, 3).reshape(S, D)
    if _trace:
        kernel._last_result = res
    return y


# revision 24
# speedup vs baseline: 1.2293x; 1.0480x over previous
"""Causal GQA self-attention (B=2, S=2048, D=2048, H=16, KV=4) on 8 TRN2 cores.

Sharding: core = (b, g) with b = batch (2) x g = kv-head group (4).
Each core computes 4 q-heads / 1 kv-head for one batch and a partial
projection output [S, D] in bf16; host sums the 4 group partials per batch.

v4 = v2 (3-phase, 303us) + targeted wins:
  - startup: group-0 QKV consumed ci-major in DMA arrival order (each
    weight/x chunk feeds 4 s-tiles), q_ps for si2/3 borrow the score psum
    slots, retuned 3-queue DMA order. Cuts ~15us of PE idle at startup.
  - row-sum matmuls removed (~26us of PE streaming): probs accumulated
    per-unit on DVE into a bf16 sbuf tile (diag blocks via one strided op),
    one small ones-matmul pair per unit; acc adds delayed one block so diag
    masks aren't queued behind them on DVE.
  - projection psum pairs alternate between B banks and an S slot
    (deeper rotation, no stall on the ev-copy latency); ev DMA alternates
    sync/gpsimd queues; 6 ev buffers.
"""
import os
import sys

if '/opt/trn_rl_repo' not in sys.path:
    sys.path.insert(0, '/opt/trn_rl_repo')

import numpy as np

B, S, D = 2, 2048, 2048
NH_TOT, NKV_TOT, HD = 16, 4, 128
NH = 4                 # q heads per core
NT = S // 128          # 16 s-tiles
NC_ = D // 128         # 16 c-tiles
T = 4                  # q-slices of 512
SM = 1.0 / np.sqrt(HD)
EPS = float(np.finfo(np.float32).eps)
ROPE_BASE = 10000.0

_PROG = None


def _build_program():
    import concourse.bass as bass
    import concourse.mybir as mybir
    import concourse.tile as tile
    from concourse import bacc
    from concourse.alu_op_type import AluOpType

    F32 = mybir.dt.float32
    BF16 = mybir.dt.bfloat16
    AF = mybir.ActivationFunctionType

    nc = bacc.Bacc("TRN2", target_bir_lowering=False, debug=False)

    XT = nc.dram_tensor("XT", [4, 4, 128, 4, 512], BF16, kind="ExternalInput")
    WQ = nc.dram_tensor("WQ", [4, 128, 4, 512], BF16, kind="ExternalInput")   # [c4, c_p, a, dq]
    WKV = nc.dram_tensor("WKV", [4, 128, 4, 256], BF16, kind="ExternalInput")
    WP = nc.dram_tensor("WP", [4, 128, NH, 512], BF16, kind="ExternalInput")  # [dq, c_in_head, h, dout]
    CS = nc.dram_tensor("CS", [2, 128, NT, HD], BF16, kind="ExternalInput")   # cos;sin
    GSM = nc.dram_tensor("GSM", [1, NH], F32, kind="ExternalInput")           # gain*sm per head
    CONST3 = nc.dram_tensor("CONST3", [128, 3, 128], BF16, kind="ExternalInput")  # ident|onesq|tri
    Y = nc.dram_tensor("Y", [NT, 2, 128, 1024], BF16, kind="ExternalOutput")

    with tile.TileContext(nc) as tc:
        with (
            tc.tile_pool(name="const", bufs=1) as const,
            tc.tile_pool(name="w", bufs=4) as wpool,
            tc.tile_pool(name="stream", bufs=3) as stream,
            tc.tile_pool(name="small", bufs=3) as small,
            tc.tile_pool(name="norm", bufs=4) as normp,
            tc.tile_pool(name="rope", bufs=2) as ropep,
            tc.tile_pool(name="qsb", bufs=4) as qsbp,
            tc.tile_pool(name="big", bufs=1) as big,
            tc.tile_pool(name="yt", bufs=2) as ytp,
            tc.tile_pool(name="probs", bufs=4) as probsp,
            tc.tile_pool(name="accp", bufs=2) as accp,
            tc.tile_pool(name="outsb", bufs=6) as outsb,
            tc.tile_pool(name="psA", bufs=2, space="PSUM") as psA,
            tc.tile_pool(name="psB", bufs=2, space="PSUM") as psB,
            tc.tile_pool(name="psS", bufs=2, space="PSUM") as psS,
        ):
            wq_sb = [wpool.tile([128, 4, 512], BF16, tag="wq", name=f"wq{i}")
                     for i in range(4)]
            wkv_sb = [wpool.tile([128, 4, 256], BF16, tag="wkv", name=f"wkv{i}")
                      for i in range(4)]
            wp = [wpool.tile([128, NH, 512], BF16, tag="wp", name=f"wp{i}")
                  for i in range(4)]
            xsp = [[stream.tile([128, 4, 512], BF16, tag="xs", bufs=12,
                                name=f"xs_{s4}_{c4}")
                    for c4 in range(4)] for s4 in range(4)]

            gsm = const.tile([1, NH], F32)
            const3 = const.tile([128, 3, 128], BF16)
            cs = const.tile([128, 2, NT, HD], BF16)

            # scalar queue: gsm, WQ0, WKV0, WQ2, WKV2, cs, const3, WP
            nc.scalar.dma_start(gsm[:], GSM[:])
            nc.scalar.dma_start(wq_sb[0][:], WQ[0])
            nc.scalar.dma_start(wkv_sb[0][:], WKV[0])
            nc.scalar.dma_start(wq_sb[2][:], WQ[2])
            nc.scalar.dma_start(wkv_sb[2][:], WKV[2])
            nc.scalar.dma_start(cs[:, 0], CS[0])
            nc.scalar.dma_start(cs[:, 1], CS[1])
            nc.scalar.dma_start(const3[:], CONST3[:])
            for dq in range(4):
                nc.scalar.dma_start(wp[dq][:], WP[dq])

            # sync queue: XT00, XT01, WQ3, WKV3, XT1 c0/c1, then Y duty
            nc.sync.dma_start(xsp[0][0][:], XT[0, 0])
            nc.sync.dma_start(xsp[0][1][:], XT[0, 1])
            nc.sync.dma_start(wq_sb[3][:], WQ[3])
            nc.sync.dma_start(wkv_sb[3][:], WKV[3])
            nc.sync.dma_start(xsp[1][0][:], XT[1, 0])
            nc.sync.dma_start(xsp[1][1][:], XT[1, 1])
            nc.sync.dma_start(xsp[2][2][:], XT[2, 2])
            nc.sync.dma_start(xsp[2][3][:], XT[2, 3])
            nc.sync.dma_start(xsp[3][2][:], XT[3, 2])
            nc.sync.dma_start(xsp[3][3][:], XT[3, 3])

            # gpsimd queue: WQ1, WKV1, XT02, XT03, XT1 c2/c3, XT2, XT3, Y duty
            nc.gpsimd.dma_start(wq_sb[1][:], WQ[1])
            nc.gpsimd.dma_start(wkv_sb[1][:], WKV[1])
            nc.gpsimd.dma_start(xsp[0][2][:], XT[0, 2])
            nc.gpsimd.dma_start(xsp[0][3][:], XT[0, 3])
            nc.gpsimd.dma_start(xsp[1][2][:], XT[1, 2])
            nc.gpsimd.dma_start(xsp[1][3][:], XT[1, 3])
            for c4 in range(2):
                nc.gpsimd.dma_start(xsp[2][c4][:], XT[2, c4])
            for c4 in range(2):
                nc.gpsimd.dma_start(xsp[3][c4][:], XT[3, c4])

            ident = const3[:, 0, :]
            onesq = const3[:, 1, :]
            tri = const3[:, 2, :]
            cos2 = cs[:, 0]
            sin2 = cs[:, 1]
            gsm_bc = const.tile([128, NH], F32)
            nc.gpsimd.partition_broadcast(gsm_bc[:], gsm[:])

            qT = big.tile([128, NH, S], BF16)
            kT = big.tile([128, S], BF16)
            v_nat = big.tile([128, NT, HD], BF16)

            # ---------------- norm + rope for one s-tile; q_src/k_src may be
            # PSUM (steady state) or SBUF (evacuated group 0)
            pending_tp = []

            def flush_tp():
                if pending_tp:
                    pending_tp.pop(0)()

            def emit_normrope(si, q_src, k_src, emit_vcopy, v_src):
                # q norm chain: ACT squares + accum
                scr = small.tile([128, 128], F32, tag="scr")
                ssq = small.tile([128, 8], F32, tag="ssq")
                for h in range(NH):
                    nc.scalar.activation(scr[:], q_src[:, h * 128:(h + 1) * 128],
                                         AF.Square, accum_out=ssq[:, h:h + 1])
                mn = small.tile([128, 8], F32, tag="mn")
                nc.vector.tensor_scalar(mn[:, 0:4], ssq[:, 0:4], 1.0 / HD, EPS,
                                        AluOpType.mult, AluOpType.add)
                rt = small.tile([128, 8], F32, tag="rt")
                nc.scalar.sqrt(rt[:, 0:4], mn[:, 0:4])
                rn = small.tile([128, 8], F32, tag="rn")
                nc.vector.reciprocal(rn[:, 0:4], rt[:, 0:4])
                qsc = small.tile([128, 4], F32, tag="qsc")
                nc.vector.tensor_tensor(qsc[:], rn[:, 0:4], gsm_bc[:], AluOpType.mult)

                qs = ropep.tile([128, 4, 128], F32, tag="qs")
                qsc_b = qsc[:, 0:4].unsqueeze(2).broadcast_to([128, 4, 128])
                q3 = q_src.rearrange("p (h d) -> p h d", h=4)
                nc.vector.tensor_tensor(qs[:], q3, qsc_b, AluOpType.mult)
                cos_b = cos2[:, si, :].unsqueeze(1).broadcast_to([128, 4, 128])
                tcs = ropep.tile([128, 4, 128], F32, tag="tcs")
                nc.vector.tensor_tensor(tcs[:], qs[:], cos_b, AluOpType.mult)
                tsn = ropep.tile([128, 4, 128], F32, tag="tsn")
                sinA = sin2[:, si, 0:64].unsqueeze(1).broadcast_to([128, 4, 64])
                sinB = sin2[:, si, 64:128].unsqueeze(1).broadcast_to([128, 4, 64])
                nc.vector.tensor_tensor(tsn[:, :, 0:64], qs[:, :, 64:128], sinA,
                                        AluOpType.mult)
                nc.vector.tensor_tensor(tsn[:, :, 64:128], qs[:, :, 0:64], sinB,
                                        AluOpType.mult)
                natq = ropep.tile([128, 4, 128], BF16, tag="natq", bufs=6)
                nc.gpsimd.tensor_tensor(natq[:], tcs[:], tsn[:], AluOpType.add)

                # k norm chain
                nc.scalar.activation(scr[:], k_src, AF.Square,
                                     accum_out=ssq[:, 4:5])
                nc.vector.tensor_scalar(mn[:, 4:5], ssq[:, 4:5], 1.0 / HD, EPS,
                                        AluOpType.mult, AluOpType.add)
                nc.scalar.sqrt(rt[:, 4:5], mn[:, 4:5])
                nc.vector.reciprocal(rn[:, 4:5], rt[:, 4:5])
                kcs = ropep.tile([128, 128], F32, tag="kcs")
                ksn = ropep.tile([128, 128], F32, tag="ksn")
                nc.vector.scalar_tensor_tensor(
                    kcs[:], k_src, rn[:, 4:5], cos2[:, si, :], AluOpType.mult,
                    AluOpType.mult)
                nc.vector.scalar_tensor_tensor(
                    ksn[:, 0:64], k_src[:, 64:128], rn[:, 4:5], sin2[:, si, 0:64],
                    AluOpType.mult, AluOpType.mult)
                nc.vector.scalar_tensor_tensor(
                    ksn[:, 64:128], k_src[:, 0:64], rn[:, 4:5], sin2[:, si, 64:128],
                    AluOpType.mult, AluOpType.mult)
                natk = ropep.tile([128, 128], BF16, tag="natk", bufs=6)
                nc.gpsimd.tensor_tensor(natk[:], kcs[:], ksn[:], AluOpType.add)

                if emit_vcopy:
                    nc.vector.tensor_copy(v_nat[:, si, :], v_src)

                def tp_(si=si, natq=natq, natk=natk):
                    for h in range(NH):
                        tp = psS.tile([128, 128], BF16, tag="S", name=f"tpq_{si}_{h}")
                        nc.tensor.transpose(tp[:], natq[:, h, :], ident[:])
                        nc.vector.tensor_copy(qT[:, h, si * 128:(si + 1) * 128], tp[:])
                    tp = psS.tile([128, 128], BF16, tag="S", name=f"tpk_{si}")
                    nc.tensor.transpose(tp[:], natk[:], ident[:])
                    nc.vector.tensor_copy(kT[:, si * 128:(si + 1) * 128], tp[:])
                pending_tp.append(tp_)

            # ---------------- phase 1a: group 0 ci-major, ACT-evacuated
            q_ps0 = [psA.tile([128, 512], F32, tag="A", name="qps0_0"),
                     psA.tile([128, 512], F32, tag="A", name="qps0_1"),
                     psS.tile([128, 512], F32, tag="S", name="qps0_2"),
                     psS.tile([128, 512], F32, tag="S", name="qps0_3")]
            kvp0 = [psB.tile([128, 512], F32, tag="B", name="kvp0_01"),
                    psB.tile([128, 512], F32, tag="B", name="kvp0_23")]
            for gi in range(4):
                for sl in range(4):
                    for a in range(4):
                        nc.tensor.matmul(
                            q_ps0[sl][:],
                            xsp[0][gi][:, a, sl * 128:(sl + 1) * 128],
                            wq_sb[gi][:, a, :],
                            start=(gi == 0 and a == 0), stop=(gi == 3 and a == 3),
                            skip_group_check=True)
                for sl in range(4):
                    kvp = kvp0[sl // 2]
                    koff = (sl % 2) * 256
                    for a in range(4):
                        # start=True zeroes the whole bank: only its first
                        # matmul sets it; odd region accumulates on zeros
                        nc.tensor.matmul(
                            kvp[:, koff:koff + 256],
                            xsp[0][gi][:, a, sl * 128:(sl + 1) * 128],
                            wkv_sb[gi][:, a, :],
                            start=(gi == 0 and a == 0 and sl % 2 == 0),
                            stop=(gi == 3 and a == 3 and sl % 2 == 1),
                            skip_group_check=True)
            for sl in range(4):
                kvp = kvp0[sl // 2]
                koff = (sl % 2) * 256
                q_sb = qsbp.tile([128, 512], F32, tag="qsb", name=f"qsb{sl}")
                nc.scalar.copy(q_sb[:], q_ps0[sl][:])
                k_sb = qsbp.tile([128, 128], F32, tag="ksb", name=f"ksb{sl}")
                nc.scalar.copy(k_sb[:], kvp[:, koff:koff + 128])
                nc.scalar.copy(v_nat[:, sl, :], kvp[:, koff + 128:koff + 256])
                emit_normrope(sl, q_sb[:], k_sb[:], False, None)

            # ---------------- phase 1b: si-major for si 4..15
            def emit_qkv(si):
                g, sl = si // 4, si % 4
                q_ps = psA.tile([128, 512], F32, tag="A", name=f"qps{si}")
                kv_ps = psB.tile([128, 256], F32, tag="B", name=f"kvps{si}")
                for i in range(NC_):
                    nc.tensor.matmul(
                        q_ps[:], xsp[g][i // 4][:, i % 4, sl * 128:(sl + 1) * 128],
                        wq_sb[i // 4][:, i % 4, :],
                        start=(i == 0), stop=(i == NC_ - 1))
                for i in range(NC_):
                    nc.tensor.matmul(
                        kv_ps[:], xsp[g][i // 4][:, i % 4, sl * 128:(sl + 1) * 128],
                        wkv_sb[i // 4][:, i % 4, :],
                        start=(i == 0), stop=(i == NC_ - 1))
                emit_normrope(si, q_ps[:], kv_ps[:, 0:128], True, kv_ps[:, 128:256])
                flush_tp()

            for si in range(4, NT):
                emit_qkv(si)

            # ---------------- phase 2: attention (no row-sum matmuls) + per-t
            # projection
            pending_rs = [None]
            pending_epi = [None]
            proj_work = []

            def gen_proj(t, yt_t):
                for si in range(4 * t, 4 * t + 4):
                    sl = si - 4 * t
                    for dtp in range(2):
                        def unit(si=si, sl=sl, dtp=dtp, yt_t=yt_t):
                            pj0 = psB.tile([128, 512], F32, tag="B",
                                           name=f"pj_{si}_{dtp}_0")
                            pj1 = psB.tile([128, 512], F32, tag="B",
                                           name=f"pj_{si}_{dtp}_1")
                            for h in range(NH):
                                lhs = yt_t[:, h, sl * 128:(sl + 1) * 128]
                                nc.tensor.matmul(pj0[:], lhs, wp[2 * dtp][:, h, :],
                                                 start=(h == 0),
                                                 stop=(h == NH - 1),
                                                 skip_group_check=True)
                                nc.tensor.matmul(pj1[:], lhs,
                                                 wp[2 * dtp + 1][:, h, :],
                                                 start=(h == 0),
                                                 stop=(h == NH - 1),
                                                 skip_group_check=True)
                            ev = outsb.tile([128, 1024], BF16, tag="ev",
                                            name=f"ev_{si}_{dtp}")
                            nc.scalar.copy(ev[:, 0:512], pj0[:])
                            nc.vector.tensor_copy(ev[:, 512:1024], pj1[:])
                            if dtp == 0:
                                nc.sync.dma_start(Y[si, dtp], ev[:])
                            else:
                                nc.gpsimd.dma_start(Y[si, dtp], ev[:])
                        proj_work.append(unit)

            def proj_step(n=1):
                for _ in range(n):
                    if proj_work:
                        proj_work.pop(0)()

            pv_ctr = [0]

            def flush_rs():
                if pending_rs[0] is not None:
                    pending_rs[0]()
                    pending_rs[0] = None

            def flush_epi():
                if pending_epi[0] is not None:
                    pending_epi[0]()
                    pending_epi[0] = None

            for t in range(T):
                yt_t = ytp.tile([128, NH, 512], BF16, tag="yt", name=f"yt{t}")
                nblk = 4 * t + 4
                for hp in (0, 2):
                    o_ps = {}
                    for h in (hp, hp + 1):
                        o_ps[h] = psA.tile([128, 512], F32, tag="A",
                                           name=f"o_ps_{t}_{h}")
                    acc = accp.tile([128, 1024], BF16, tag="acc",
                                    name=f"acc_{t}_{hp}")
                    prb_tiles = {}
                    pending_add = [None]

                    def emit_sc(j, t=t, hp=hp, acc=acc, prb_tiles=prb_tiles,
                                pending_add=pending_add):
                        off = j - 4 * t
                        q0 = max(off, 0) * 128
                        sc = psS.tile([128, 1024], F32, tag="S",
                                      name=f"sc_{t}_{hp}_{j}")
                        for u, h in enumerate((hp, hp + 1)):
                            nc.tensor.matmul(
                                sc[:, u * 512 + q0:(u + 1) * 512],
                                kT[:, j * 128:(j + 1) * 128],
                                qT[:, h, t * 512 + q0:(t + 1) * 512],
                                start=True, stop=True, skip_group_check=True)
                        prb = probsp.tile([128, 1024], BF16, tag="probs",
                                          name=f"prb_{t}_{hp}_{j}")
                        if off <= 0:
                            nc.scalar.activation(prb[:], sc[:], AF.Exp)
                        else:
                            for u in range(2):
                                nc.scalar.activation(
                                    prb[:, u * 512 + q0:(u + 1) * 512],
                                    sc[:, u * 512 + q0:(u + 1) * 512], AF.Exp)
                        if off >= 0:
                            pr2 = prb[:].rearrange("p (u c) -> p u c", u=2)
                            tri_b = tri[:].unsqueeze(1).broadcast_to([128, 2, 128])
                            nc.vector.tensor_tensor(pr2[:, :, q0:q0 + 128],
                                                    pr2[:, :, q0:q0 + 128],
                                                    tri_b, AluOpType.mult)
                        # acc add delayed one block so the next diag mask isn't
                        # queued behind it on DVE
                        if pending_add[0] is not None:
                            pending_add[0]()

                        def mk_add(j=j, off=off, q0=q0, prb=prb, acc=acc):
                            def f():
                                if j == 0:
                                    nc.vector.tensor_copy(acc[:], prb[:])
                                elif off <= 0:
                                    nc.vector.tensor_tensor(
                                        acc[:], acc[:], prb[:], AluOpType.add)
                                else:
                                    ac2 = acc[:].rearrange("p (u c) -> p u c", u=2)
                                    pr2 = prb[:].rearrange("p (u c) -> p u c", u=2)
                                    nc.vector.tensor_tensor(
                                        ac2[:, :, q0:512], ac2[:, :, q0:512],
                                        pr2[:, :, q0:512], AluOpType.add)
                            return f
                        pending_add[0] = mk_add()
                        prb_tiles[j] = prb

                    def emit_pv(j, t=t, hp=hp, o_ps=o_ps, prb_tiles=prb_tiles,
                                nblk=nblk):
                        off = j - 4 * t
                        q0 = max(off, 0) * 128
                        prb = prb_tiles.pop(j)
                        first = (j == 0)
                        last = (j == nblk - 1)
                        for u, h in enumerate((hp, hp + 1)):
                            nc.tensor.matmul(
                                o_ps[h][:, q0:512], v_nat[:, j, :],
                                prb[:, u * 512 + q0:(u + 1) * 512],
                                start=first, stop=last, skip_group_check=True)
                        pv_ctr[0] += 1
                        if pv_ctr[0] % 2 == 0:
                            proj_step(1)

                    depth = 3
                    for j in range(depth):
                        emit_sc(j)
                        if j == 1:
                            flush_rs()
                    for j in range(depth, nblk):
                        emit_sc(j)
                        emit_pv(j - depth)
                        if j == depth:
                            flush_epi()
                    for j in range(nblk - depth, nblk):
                        emit_pv(j)
                    if pending_add[0] is not None:
                        pending_add[0]()
                        pending_add[0] = None
                    flush_tp()   # drain leftover transpose groups (si 12..15)

                    rs_cell = {}

                    def make_rs(hp=hp, acc=acc, rs_cell=rs_cell, t=t):
                        def f():
                            for u, h in enumerate((hp, hp + 1)):
                                r = psB.tile([128, 512], F32, tag="B",
                                             name=f"rs_{t}_{h}")
                                nc.tensor.matmul(r[:], onesq[:],
                                                 acc[:, u * 512:(u + 1) * 512],
                                                 start=True, stop=True,
                                                 skip_group_check=True)
                                rs_cell[h] = r
                        return f

                    def make_epi(hp=hp, o_ps=o_ps, rs_cell=rs_cell, yt_t=yt_t):
                        def f():
                            rcp0 = normp.tile([128, 512], F32, tag="rcp")
                            rcp1 = normp.tile([128, 512], F32, tag="rcp")
                            nc.vector.reciprocal_approx_fast(rcp0[:], rs_cell[hp][:])
                            nc.vector.reciprocal_approx_fast(rcp1[:],
                                                             rs_cell[hp + 1][:])
                            nc.vector.tensor_tensor(
                                yt_t[:, hp, :], o_ps[hp][:], rcp0[:],
                                AluOpType.mult)
                            nc.vector.tensor_tensor(
                                yt_t[:, hp + 1, :], o_ps[hp + 1][:], rcp1[:],
                                AluOpType.mult)
                        return f

                    pending_rs[0] = make_rs()
                    pending_epi[0] = make_epi()

                # flush the unit's rs+epilogue, then queue this group's
                # projection; it drains as PE filler inside the (ACT-paced)
                # next attention window
                flush_rs()
                flush_epi()
                proj_step(len(proj_work))   # finish any leftover prior group
                gen_proj(t, yt_t)
            proj_step(len(proj_work))       # drain proj(3)

    nc.compile()
    return nc


def _host_inputs(x, Wq, Wk, Wv, Wproj, q_gain):
    import ml_dtypes
    bf16 = ml_dtypes.bfloat16

    x = np.asarray(x, dtype=np.float32)
    Wq = np.asarray(Wq, dtype=np.float32)
    Wk = np.asarray(Wk, dtype=np.float32)
    Wv = np.asarray(Wv, dtype=np.float32)
    Wproj = np.asarray(Wproj, dtype=np.float32)
    q_gain = np.asarray(q_gain, dtype=np.float32)

    inv = (1.0 / ROPE_BASE ** (np.arange(0, HD, 2, dtype=np.float32) / HD)).astype(np.float32)
    ang = np.outer(np.arange(S, dtype=np.float32), inv)
    cos = np.cos(ang).astype(np.float32)
    sin = np.sin(ang).astype(np.float32)
    cos2 = np.concatenate([cos, cos], 1).reshape(NT, 128, HD).transpose(1, 0, 2)
    sin2 = np.concatenate([sin, -sin], 1).reshape(NT, 128, HD).transpose(1, 0, 2)
    cs = np.ascontiguousarray(np.stack([cos2, sin2])).astype(bf16)  # [2,128,NT,HD]

    qq = np.arange(128)[None, :]
    kk = np.arange(128)[:, None]
    const3 = np.ascontiguousarray(np.stack(
        [np.eye(128, dtype=np.float32),
         np.ones((128, 128), dtype=np.float32),
         (kk <= qq).astype(np.float32)], axis=1)).astype(bf16)  # [128,3,128]

    xTb = [np.ascontiguousarray(
        x[b].T.astype(bf16).reshape(4, 4, 128, 4, 512).transpose(3, 0, 2, 1, 4))
        for b in range(B)]

    in_maps = []
    for cid in range(8):
        b, g = cid // 4, cid % 4
        wq = Wq[g * 512:(g + 1) * 512, :].T            # [D, 512]
        wk = Wk[g * 128:(g + 1) * 128, :].T            # [D, 128]
        wv = Wv[g * 128:(g + 1) * 128, :].T
        wkv = np.concatenate([wk, wv], 1)              # [D, 256]
        wp = Wproj[:, g * 512:(g + 1) * 512].T         # [512, D] (c_local, dout)
        in_maps.append({
            "XT": xTb[b],
            "WQ": np.ascontiguousarray(
                wq.reshape(4, 4, 128, 512).transpose(0, 2, 1, 3)).astype(bf16),
            "WKV": np.ascontiguousarray(
                wkv.reshape(4, 4, 128, 256).transpose(0, 2, 1, 3)).astype(bf16),
            "WP": np.ascontiguousarray(
                wp.reshape(NH, 128, 4, 512).transpose(2, 1, 0, 3)).astype(bf16),
            "CS": cs,
            "GSM": (q_gain[g * 4:(g + 1) * 4] * SM).reshape(1, NH).astype(np.float32),
            "CONST3": const3,
        })
    return in_maps


def _get_prog():
    global _PROG
    if _PROG is None:
        _PROG = _build_program()
    return _PROG


def kernel(x, Wq, Wk, Wv, Wproj, q_gain, _trace=False, _tmpdir=None):
    from concourse.bass_utils import run_bass_kernel_spmd
    nc = _get_prog()
    in_maps = _host_inputs(x, Wq, Wk, Wv, Wproj, q_gain)
    kwargs = {}
    if _tmpdir is not None:
        os.makedirs(_tmpdir, exist_ok=True)
        kwargs["tmpdir"] = _tmpdir
    res = run_bass_kernel_spmd(nc, in_maps, list(range(8)), trace=_trace, **kwargs)
    y = np.empty((B, S, D), dtype=np.float32)
    for b in range(B):
        acc = res.results[4 * b]["Y"].astype(np.float32)
        for g in range(1, 4):
            acc = acc + res.results[4 * b + g]["Y"].astype(np.float32)
        y[b] = acc.transpose(0, 2, 1, 3).reshape(S, D)
    if _trace:
        kernel._last_result = res
    return y
